# revision 39
# baseline (speedup 1.0000x reference)
"""MultiHeadAttention Trainium2 kernel.

Problem (hardcoded): S=2048, B=2, D=1024, H=16, HD=64, fp32 I/O.
  q = query @ w_q.T + b_q   (same for k, v), heads split from D
  scores[i,j,b,h] = (q_i . k_j)/8, masked where mask[j]==0, softmax over j
  out[i,b,:] = concat_h( sum_j p_ij v_j )

Sharding: 8 cores = 2 batches x 4 head-groups (4 heads / 256 dims each).
Host-side prep: cast to bf16, transpose to [D, seq] layout, and compact the
key/value sequence to the unmasked positions only (masked j contribute
exactly 0 after softmax), padded to a multiple of 128.

Per-core program (Tile framework), engineered so the ACT engine (exp) is
saturated from ~t=20us on and PE work hides beneath it:
  - Q,K projections k-outer (start as soon as the first k-tile lands),
    output qT/kT[o, i] with o (head dims) on partitions, bf16.
  - V projection flipped (x^T tiles stationary) giving V[j, o] with j on
    partitions; V_ext adds a per-head mask column (1 real / 0 padding) so
    the softmax denominator falls out of the PV matmul.
  - Scores transposed: S^T[j, i] = kT.T @ qT, two heads packed in the
    128-row PE array via base-partition row tiling (d=64 each).
  - P^T = exp(S^T / 8) on ACT in [128, 1024] chunks (bf16 out).
  - PV: out^T[vd, i] += V_ext[j,:].T @ P^T[j, i-chunk]  (stationary V_ext
    is tiny -> no LDWEIGHTS bottleneck), fp32 PSUM accumulation over j.
  - Normalize: reciprocal of denominator row, PE-broadcast (K=1 matmul,
    float32r), DVE multiply; output stays [vd, i] and the host transposes.
"""

import math
import os
import sys

sys.path.insert(0, "/opt/trn_rl_repo")

import numpy as np
import ml_dtypes

import concourse.bass as bass
import concourse.tile as tile
from concourse import bacc, mybir
from concourse.bass_utils import run_bass_kernel_spmd

S, B, D, H, HD = 2048, 2, 1024, 16, 64
N_CORES = 8
GROUPS = 4          # head groups (cores per batch)
GH = H // GROUPS    # heads per core = 4
GD = GH * HD        # dims per core = 256
KT = D // 128       # contraction k-tiles = 8
IBLK = 1024         # i block (exp granularity / P^T tile width)
NIB = S // IBLK     # i blocks = 2
VW1 = HD + 1        # per-head vext width (64 v cols + denominator col)

BF16 = mybir.dt.bfloat16
F32 = mybir.dt.float32
F32R = mybir.dt.float32r
EXP = mybir.ActivationFunctionType.Exp

_CACHE = {}


def _chunks(total, step):
    out = []
    o = 0
    while o < total:
        n = min(step, total - o)
        out.append((o, n))
        o += n
    return out


def _pairs(seq):
    return [tuple(seq[i:i + 2]) for i in range(0, len(seq), 2)]


def _build(J, J_real, use_bias):
    """Build + compile the per-core Bass program (identical on all cores)."""
    NJT = J // 128
    nc = bacc.Bacc("TRN2", target_bir_lowering=False, debug=False,
                   enable_asserts=False)

    xq_d = nc.dram_tensor("xq", (D, S), BF16, kind="ExternalInput")
    xk_d = nc.dram_tensor("xk", (D, J), BF16, kind="ExternalInput")
    xv_d = nc.dram_tensor("xv", (D, J), BF16, kind="ExternalInput")
    wq_d = nc.dram_tensor("wq", (D, GD), BF16, kind="ExternalInput")
    wk_d = nc.dram_tensor("wk", (D, GD), BF16, kind="ExternalInput")
    wv_d = nc.dram_tensor("wv", (D, GD), BF16, kind="ExternalInput")
    mpad_d = nc.dram_tensor("mpad", (128, NJT), BF16, kind="ExternalInput")
    if use_bias:
        bq_d = nc.dram_tensor("bq", (GD, 1), F32, kind="ExternalInput")
        bk_d = nc.dram_tensor("bk", (GD, 1), F32, kind="ExternalInput")
        bv_d = nc.dram_tensor("bv", (1, GD), BF16, kind="ExternalInput")
    out_d = nc.dram_tensor("out", (GH * VW1, S), F32, kind="ExternalOutput")

    # SBUF budget for the P^T pool, in per-partition bytes (SBUF tiles
    # reserve their free-dim byte span on every partition).
    fixed_pp = (KT * S * 2                 # xq tiles
                + 2 * KT * J * 2           # xk, xv tiles
                + 3 * KT * GD * 2          # weights
                + 2 * S * 2 + 2 * J * 2    # qT/kT pool
                + NJT * (GH * VW1 + 8) * 2   # vext
                + 4 * 512 * 4              # out staging
                + 6 * 1024)                # consts, mpad, small, slack
    budget_pp = 188 * 1024 - fixed_pp
    pt_bufs = max(NJT + 2, min(4 * NJT + 4, budget_pp // (IBLK * 2)))
    # pipelined schedule holds two full score phases of P^T tiles
    pipelined = pt_bufs >= 4 * NJT

    scale = 1.0 / math.sqrt(HD)  # 0.125, folded into the exp

    with tile.TileContext(nc) as tc:
        with (
            tc.tile_pool(name="xq", bufs=KT) as xq_p,
            tc.tile_pool(name="xk", bufs=KT) as xk_p,
            tc.tile_pool(name="xv", bufs=KT) as xv_p,
            tc.tile_pool(name="w", bufs=3) as w_p,
            tc.tile_pool(name="qk", bufs=2) as qk_p,
            tc.tile_pool(name="vext", bufs=NJT) as vext_p,
            tc.tile_pool(name="pt", bufs=pt_bufs) as pt_p,
            tc.tile_pool(name="small", bufs=10) as small_p,
            tc.tile_pool(name="ost", bufs=4) as ost_p,
            tc.tile_pool(name="sps", bufs=3, space="PSUM") as sps_p,
            tc.tile_pool(name="pps", bufs=2, space="PSUM") as pps_p,
        ):
            # ---- input DMAs (ordered: K-proj needs land first) ----
            def load_w(w_d):
                w_sb = w_p.tile([128, KT * GD], BF16, tag="w", name="w_sb")
                nc.sync.dma_start(
                    w_sb[:].rearrange("p (k o) -> p k o", k=KT),
                    w_d.ap().rearrange("(k p) o -> p k o", p=128))
                return w_sb

            wk_sb = load_w(wk_d)
            wq_sb = load_w(wq_d)
            mpad_sb = small_p.tile([128, NJT], BF16, tag="mpad")
            nc.sync.dma_start(mpad_sb[:], mpad_d.ap())
            # interleave K/Q k-tiles so both projections' k-loops advance
            # together behind the DMA stream; first column-halves land first
            # so the first score phase (i 0:1024, j 0:1024) starts early.
            xk_t = []
            xq_t = []
            jh = min(1024, J)
            for k in range(KT):
                t = xk_p.tile([128, J], BF16, tag="xk", name=f"xk{k}")
                nc.sync.dma_start(t[:, 0:jh],
                                  xk_d.ap()[k * 128:(k + 1) * 128, 0:jh])
                xk_t.append(t)
                t = xq_p.tile([128, S], BF16, tag="xq", name=f"xq{k}")
                nc.sync.dma_start(t[:, 0:1024],
                                  xq_d.ap()[k * 128:(k + 1) * 128, 0:1024])
                xq_t.append(t)
            for k in range(KT):
                if J > jh:
                    nc.sync.dma_start(xk_t[k][:, jh:J],
                                      xk_d.ap()[k * 128:(k + 1) * 128, jh:J])
                nc.sync.dma_start(xq_t[k][:, 1024:S],
                                  xq_d.ap()[k * 128:(k + 1) * 128, 1024:S])
            wv_sb = load_w(wv_d)
            xv_t = []
            for k in range(KT):
                t = xv_p.tile([128, J], BF16, tag="xv", name=f"xv{k}")
                nc.sync.dma_start(t[:], xv_d.ap()[k * 128:(k + 1) * 128, :])
                xv_t.append(t)
            if use_bias:
                bq_c = small_p.tile([128, 2], F32, tag="biasq")
                nc.sync.dma_start(
                    bq_c[:].rearrange("p (o x) -> p o x", o=2),
                    bq_d.ap().rearrange("(o p) x -> p o x", p=128))
                bk_c = small_p.tile([128, 2], F32, tag="biask")
                nc.sync.dma_start(
                    bk_c[:].rearrange("p (o x) -> p o x", o=2),
                    bk_d.ap().rearrange("(o p) x -> p o x", p=128))
                bv_row = small_p.tile([1, GD], BF16, tag="bvrow")
                nc.sync.dma_start(bv_row[:], bv_d.ap())
                ones_row = small_p.tile([1, 128], BF16, tag="ones")
                nc.vector.memset(ones_row[:], 1.0)

            # prime the ACT exp table during the initial DMA window
            warm = small_p.tile([1, 8], F32, tag="warm")
            nc.vector.memset(warm[:], 0.0)
            warm2 = small_p.tile([1, 8], F32, tag="warm2")
            nc.scalar.activation(warm2[:], warm[:], EXP, scale=1.0)

            # ---- projections ----
            qT = {}   # per otile: [128, S] bf16  (o on partitions)
            kTt = {}  # per otile: [128, J] bf16

            def proj_pass(x_tiles, w_sb, dst, bias_col, ot, chunk_group):
                """One k-outer accumulation pass over <=2 width-chunks.
                The 128 output dims are column-packed as two concurrent
                M=64 matmuls (tile_position (0,0)/(0,64)) sharing one rhs
                stream via separate XBUS groups."""
                ps = [pps_p.tile([128, 512], F32, tag="pps",
                                 name=f"pps{ot}{o0}") for (o0, _) in chunk_group]
                for k in range(KT):
                    lw = w_sb[:, k * GD + ot * 128:k * GD + (ot + 1) * 128]
                    for ci, (o0, n) in enumerate(chunk_group):
                        nc.tensor.matmul(ps[ci][:, 0:n], lhsT=lw,
                                         rhs=x_tiles[k][:, o0:o0 + n],
                                         start=(k == 0), stop=(k == KT - 1))
                for ci, (o0, n) in enumerate(chunk_group):
                    if use_bias:
                        nc.vector.tensor_scalar(
                            dst[:, o0:o0 + n], ps[ci][:, 0:n],
                            bias_col[:, ot:ot + 1], None,
                            mybir.AluOpType.add)
                    else:
                        nc.vector.tensor_copy(dst[:, o0:o0 + n], ps[ci][:, 0:n])

            def proj_passes(x_tiles, w_sb, dst_map, bias_col, width, ot):
                dst = qk_p.tile([128, width], BF16,
                                tag="qt" if width == S else "kt",
                                name=f"qk{ot}")
                dst_map[ot] = dst
                return [
            # returns one closure per single-chunk pass (filler-sized)
                    (lambda cg=cg: proj_pass(x_tiles, w_sb, dst, bias_col,
                                             ot, cg))
                    for cg in [[c] for c in _chunks(width, 512)]
                ]

            vext = [None] * NJT

            def v_wave(jts):
                """V projection (flipped orientation) for a couple of j-tiles
                + V_ext assembly."""
                ps = [pps_p.tile([128, GD], F32, tag="pps", name=f"ppsv{jt}")
                      for jt in jts]
                for k in range(KT):
                    for vi, jt in enumerate(jts):
                        nc.tensor.matmul(
                            ps[vi][:, :],
                            lhsT=xv_t[k][:, jt * 128:(jt + 1) * 128],
                            rhs=wv_sb[:, k * GD:(k + 1) * GD],
                            start=(k == 0),
                            stop=(k == KT - 1) and not use_bias)
                for vi, jt in enumerate(jts):
                    if use_bias:
                        nc.tensor.matmul(ps[vi][:, :], lhsT=ones_row[:, :],
                                         rhs=bv_row[:, :], start=False,
                                         stop=True)
                    ve = vext_p.tile([128, GH * VW1], BF16, tag="vext",
                                     name=f"vext{jt}")
                    for h in range(GH):
                        nc.vector.tensor_copy(
                            ve[:, h * VW1:h * VW1 + HD],
                            ps[vi][:, h * HD:(h + 1) * HD])
                        nc.vector.tensor_copy(
                            ve[:, h * VW1 + HD:h * VW1 + HD + 1],
                            mpad_sb[:, jt:jt + 1])
                    vext[jt] = ve

            def emit_qkt(ib, hp, fillers, start_jt=3):
                """Scores + exp for head pair hp of i-block ib. Emits one
                filler closure after each j-tile (from start_jt on) to keep
                PE fed while ACT chews on the exps; fillers too early would
                instead starve the score stream."""
                i0 = ib * IBLK
                pt = {}
                for jt in range(NJT):
                    psA = sps_p.tile([128, IBLK], F32, tag="sps",
                                     name=f"sA{ib}{hp}{jt}")
                    psB = sps_p.tile([128, IBLK], F32, tag="sps",
                                     name=f"sB{ib}{hp}{jt}")
                    for (o, n) in _chunks(IBLK, 512):
                        nc.tensor.matmul(
                            psA[:, o:o + n],
                            lhsT=kTt[hp][0:64, jt * 128:(jt + 1) * 128],
                            rhs=qT[hp][0:64, i0 + o:i0 + o + n],
                            start=True, stop=True)
                        nc.tensor.matmul(
                            psB[:, o:o + n],
                            lhsT=kTt[hp][64:128, jt * 128:(jt + 1) * 128],
                            rhs=qT[hp][64:128, i0 + o:i0 + o + n],
                            start=True, stop=True)
                    ptA = pt_p.tile([128, IBLK], BF16, tag="pt",
                                    name=f"ptA{ib}{hp}{jt}")
                    ptB = pt_p.tile([128, IBLK], BF16, tag="pt",
                                    name=f"ptB{ib}{hp}{jt}")
                    nc.scalar.activation(ptA[:], psA[:], EXP, scale=scale)
                    nc.scalar.activation(ptB[:], psB[:], EXP, scale=scale)
                    pt[(hp * 2, jt)] = ptA
                    pt[(hp * 2 + 1, jt)] = ptB
                    if jt >= start_jt and fillers:
                        fillers.pop(0)()
                while fillers:
                    fillers.pop(0)()
                return pt

            def pv_group(ib, hp, hl, icl, pt):
                """One PV accumulation group; numerators + denominator row
                DMA'd out unnormalized (host folds the division into
                unsharding)."""
                if True:
                    h = hp * 2 + hl
                    if True:
                        pv = pps_p.tile([VW1, 512], F32, tag="pps",
                                        name=f"pv{ib}{h}{icl}")
                        for jt in range(NJT):
                            nc.tensor.matmul(
                                pv[:, :],
                                lhsT=vext[jt][:, h * VW1:(h + 1) * VW1],
                                rhs=pt[(h, jt)][:, icl * 512:(icl + 1) * 512],
                                start=(jt == 0), stop=(jt == NJT - 1))
                        osb = ost_p.tile([VW1, 512], F32, tag="ost",
                                         name=f"o{ib}{h}{icl}")
                        nc.vector.tensor_copy(osb[:], pv[:, :])
                        nc.sync.dma_start(
                            out_d.ap()[h * VW1:(h + 1) * VW1,
                                       ib * IBLK + icl * 512:
                                       ib * IBLK + (icl + 1) * 512],
                            osb[:])

            # ---- emission schedule ----
            bqc = bq_c if use_bias else None
            bkc = bk_c if use_bias else None
            k0_passes = proj_passes(xk_t, wk_sb, kTt, bkc, J, 0)
            k0_passes[0]()   # j 0:512
            if len(k0_passes) > 1:
                k0_passes[1]()   # j 512:1024
            q0_passes = proj_passes(xq_t, wq_sb, qT, bqc, S, 0)
            q0_passes[0]()
            q0_passes[1]()   # i 0:1024 -> first score block can start

            v_fillers = [(lambda js=js: v_wave(js))
                         for js in _pairs(list(range(NJT)))]
            q1_fillers = proj_passes(xq_t, wq_sb, qT, bqc, S, 1)
            k1_fillers = proj_passes(xk_t, wk_sb, kTt, bkc, J, 1)

            def pv_fillers(ib, hp, pt):
                return [(lambda hl=hl, icl=icl: pv_group(ib, hp, hl, icl, pt))
                        for hl in range(2) for icl in range(IBLK // 512)]

            def pv_tail(ib, hp, pt):
                """Final-phase PV: interleave two groups' j-loops so their
                matmuls ride under the last exp window."""
                for hl in range(2):
                    h = hp * 2 + hl
                    pvs = []
                    for icl in range(IBLK // 512):
                        pv = pps_p.tile([VW1, 512], F32, tag="pps",
                                        name=f"pvt{ib}{h}{icl}")
                        pvs.append(pv)
                    for jt in range(NJT):
                        for icl in range(IBLK // 512):
                            nc.tensor.matmul(
                                pvs[icl][:, :],
                                lhsT=vext[jt][:, h * VW1:(h + 1) * VW1],
                                rhs=pt[(h, jt)][:, icl * 512:(icl + 1) * 512],
                                start=(jt == 0), stop=(jt == NJT - 1))
                    for icl in range(IBLK // 512):
                        osb = ost_p.tile([VW1, 512], F32, tag="ost",
                                         name=f"ot{ib}{h}{icl}")
                        nc.vector.tensor_copy(osb[:], pvs[icl][:, :])
                        nc.sync.dma_start(
                            out_d.ap()[h * VW1:(h + 1) * VW1,
                                       ib * IBLK + icl * 512:
                                       ib * IBLK + (icl + 1) * 512],
                            osb[:])

            if pipelined:
                # Front = K0/Q0 first halves only; every other matmul phase
                # runs as filler inside a later score phase's exp window.
                # PV groups of phase X release P^T buffers just in time for
                # phase X+2's allocations.
                pt00 = emit_qkt(0, 0,
                                k0_passes[2:] + q0_passes[2:]
                                + q1_fillers + v_fillers[:1])
                g00 = pv_fillers(0, 0, pt00)
                pt10 = emit_qkt(1, 0, k1_fillers + v_fillers[1:],
                                start_jt=2)
                g10 = pv_fillers(1, 0, pt10)
                pt01 = emit_qkt(0, 1, g00 + g10[:2], start_jt=1)
                g01 = pv_fillers(0, 1, pt01)
                pt11 = emit_qkt(1, 1, g10[2:] + g01, start_jt=1)
                pv_tail(1, 1, pt11)
            else:
                for p in k0_passes[2:] + q0_passes[2:]:
                    p()
                pt00 = emit_qkt(0, 0, [])
                for f in v_fillers + q1_fillers + k1_fillers:
                    f()
                for f in pv_fillers(0, 0, pt00):
                    f()
                pt10 = emit_qkt(1, 0, [])
                for f in pv_fillers(1, 0, pt10):
                    f()
                pt01 = emit_qkt(0, 1, [])
                for f in pv_fillers(0, 1, pt01):
                    f()
                pt11 = emit_qkt(1, 1, [])
                pv_tail(1, 1, pt11)

    nc.compile()
    return nc


def _prep_and_run(inputs, trace=False):
    query = np.asarray(inputs["query"], dtype=np.float32)
    key = np.asarray(inputs["key"], dtype=np.float32)
    value = np.asarray(inputs["value"], dtype=np.float32)
    mask = np.asarray(inputs["mask"]).reshape(S)
    w_q = np.asarray(inputs["w_q"], dtype=np.float32)
    b_q = np.asarray(inputs["b_q"], dtype=np.float32)
    w_k = np.asarray(inputs["w_k"], dtype=np.float32)
    b_k = np.asarray(inputs["b_k"], dtype=np.float32)
    w_v = np.asarray(inputs["w_v"], dtype=np.float32)
    b_v = np.asarray(inputs["b_v"], dtype=np.float32)

    use_bias = bool(np.any(b_q) or np.any(b_k) or np.any(b_v))

    # compact key/value over masked-out positions
    idx = np.nonzero(mask != 0)[0]
    J_real = int(len(idx))
    assert J_real > 0, "all positions masked: softmax undefined"
    J = max(512, ((J_real + 127) // 128) * 128)
    key_c = np.zeros((J, B, D), np.float32)
    key_c[:J_real] = key[idx]
    value_c = np.zeros((J, B, D), np.float32)
    value_c[:J_real] = value[idx]

    bf = ml_dtypes.bfloat16
    NJT = J // 128
    mflat = np.zeros(J, np.float32)
    mflat[:J_real] = 1  # mpad[p, t] = 1 iff t*128+p < J_real
    mpad = np.ascontiguousarray(mflat.reshape(NJT, 128).T).astype(bf)
    in_maps = []
    for core in range(N_CORES):
        b = core // GROUPS
        g = core % GROUPS
        hs = slice(g * GD, (g + 1) * GD)
        m = {
            "xq": np.ascontiguousarray(query[:, b, :].T).astype(bf),
            "xk": np.ascontiguousarray(key_c[:, b, :].T).astype(bf),
            "xv": np.ascontiguousarray(value_c[:, b, :].T).astype(bf),
            "wq": np.ascontiguousarray(w_q[hs, :].T).astype(bf),
            "wk": np.ascontiguousarray(w_k[hs, :].T).astype(bf),
            "wv": np.ascontiguousarray(w_v[hs, :].T).astype(bf),
            "mpad": mpad,
        }
        if use_bias:
            m["bq"] = np.ascontiguousarray(b_q[hs]).reshape(GD, 1)
            m["bk"] = np.ascontiguousarray(b_k[hs]).reshape(GD, 1)
            m["bv"] = np.ascontiguousarray(b_v[hs]).reshape(1, GD).astype(bf)
        in_maps.append(m)

    ck = (J, J_real, use_bias)
    if ck not in _CACHE:
        _CACHE[ck] = _build(J, J_real, use_bias)
    nc = _CACHE[ck]

    kwargs = {}
    if trace:
        kwargs = dict(trace=True, trace_cores=list(range(N_CORES)))
    res = run_bass_kernel_spmd(nc, in_maps, core_ids=list(range(N_CORES)),
                               **kwargs)

    out = np.empty((S, B, D), np.float32)
    for core in range(N_CORES):
        b = core // GROUPS
        g = core % GROUPS
        r = res.results[core]["out"].reshape(GH, VW1, S)
        out[:, b, g * GD:(g + 1) * GD] = (
            (r[:, :HD, :] / r[:, HD:HD + 1, :])     # softmax denominator
            .reshape(GD, S).T)
    return out, res


def kernel(**inputs):
    out, _ = _prep_and_run(inputs, trace=False)
    return out


def run_traced(**inputs):
    _, res = _prep_and_run(inputs, trace=True)
    return res


# revision 40
# speedup vs baseline: 1.1763x; 1.1763x over previous
"""MultiHeadAttention Trainium2 kernel.

Problem (hardcoded): S=2048, B=2, D=1024, H=16, HD=64, fp32 I/O.
  q = query @ w_q.T + b_q   (same for k, v), heads split from D
  scores[i,j,b,h] = (q_i . k_j)/8, masked where mask[j]==0, softmax over j
  out[i,b,:] = concat_h( sum_j p_ij v_j )

Sharding: 8 cores = 2 batches x 4 head-groups (4 heads / 256 dims each).
Host-side prep: cast to bf16, transpose to [D, seq] layout, and compact the
key/value sequence to the unmasked positions only (masked j contribute
exactly 0 after softmax), padded to a multiple of 128.

Per-core program (Tile framework), engineered so the ACT engine (exp) is
saturated from ~t=20us on and PE work hides beneath it:
  - Q,K projections k-outer (start as soon as the first k-tile lands),
    output qT/kT[o, i] with o (head dims) on partitions, bf16.
  - V projection flipped (x^T tiles stationary) giving V[j, o] with j on
    partitions; V_ext adds a per-head mask column (1 real / 0 padding) so
    the softmax denominator falls out of the PV matmul.
  - Scores transposed: S^T[j, i] = kT.T @ qT, two heads packed in the
    128-row PE array via base-partition row tiling (d=64 each).
  - P^T = exp(S^T / 8) on ACT in [128, 1024] chunks (bf16 out).
  - PV: out^T[vd, i] += V_ext[j,:].T @ P^T[j, i-chunk]  (stationary V_ext
    is tiny -> no LDWEIGHTS bottleneck), fp32 PSUM accumulation over j.
  - Normalize: reciprocal of denominator row, PE-broadcast (K=1 matmul,
    float32r), DVE multiply; output stays [vd, i] and the host transposes.
"""

import math
import os
import sys

sys.path.insert(0, "/opt/trn_rl_repo")

import numpy as np
import ml_dtypes

import concourse.bass as bass
import concourse.tile as tile
from concourse import bacc, mybir
from concourse.bass_utils import run_bass_kernel_spmd

S, B, D, H, HD = 2048, 2, 1024, 16, 64
N_CORES = 8
GROUPS = 4          # head groups (cores per batch)
GH = H // GROUPS    # heads per core = 4
GD = GH * HD        # dims per core = 256
KT = D // 128       # contraction k-tiles = 8
IBLK = 1024         # i block (exp granularity / P^T tile width)
NIB = S // IBLK     # i blocks = 2
VW1 = HD + 1        # per-head vext width (64 v cols + denominator col)

BF16 = mybir.dt.bfloat16
F32 = mybir.dt.float32
F32R = mybir.dt.float32r
EXP = mybir.ActivationFunctionType.Exp

_CACHE = {}


def _chunks(total, step):
    out = []
    o = 0
    while o < total:
        n = min(step, total - o)
        out.append((o, n))
        o += n
    return out


def _pairs(seq):
    return [tuple(seq[i:i + 2]) for i in range(0, len(seq), 2)]


def _build(J, J_real, use_bias):
    """Build + compile the per-core Bass program (identical on all cores)."""
    NJT = J // 128
    nc = bacc.Bacc("TRN2", target_bir_lowering=False, debug=False,
                   enable_asserts=False)

    xq_d = nc.dram_tensor("xq", (D, S), BF16, kind="ExternalInput")
    xk_d = nc.dram_tensor("xk", (D, J), BF16, kind="ExternalInput")
    xv_d = nc.dram_tensor("xv", (D, J), BF16, kind="ExternalInput")
    wq_d = nc.dram_tensor("wq", (D, GD), BF16, kind="ExternalInput")
    wk_d = nc.dram_tensor("wk", (D, GD), BF16, kind="ExternalInput")
    wv_d = nc.dram_tensor("wv", (D, GD), BF16, kind="ExternalInput")
    mpad_d = nc.dram_tensor("mpad", (128, NJT), BF16, kind="ExternalInput")
    if use_bias:
        bq_d = nc.dram_tensor("bq", (GD, 1), F32, kind="ExternalInput")
        bk_d = nc.dram_tensor("bk", (GD, 1), F32, kind="ExternalInput")
        bv_d = nc.dram_tensor("bv", (1, GD), BF16, kind="ExternalInput")
    out_d = nc.dram_tensor("out", (GH * VW1, S), F32, kind="ExternalOutput")

    # SBUF budget for the P^T pool, in per-partition bytes (SBUF tiles
    # reserve their free-dim byte span on every partition).
    fixed_pp = (KT * S * 2                 # xq tiles
                + 2 * KT * J * 2           # xk, xv tiles
                + 3 * KT * GD * 2          # weights
                + 2 * S * 2 + 2 * J * 2    # qT/kT pool
                + NJT * (GH * VW1 + 8) * 2   # vext
                + 4 * 512 * 4              # out staging
                + 6 * 1024)                # consts, mpad, small, slack
    budget_pp = 188 * 1024 - fixed_pp
    pt_bufs = max(NJT + 2, min(4 * NJT + 4, budget_pp // (IBLK * 2)))
    # pipelined schedule holds two full score phases of P^T tiles
    pipelined = pt_bufs >= 4 * NJT

    scale = 1.0 / math.sqrt(HD)  # 0.125, folded into the exp

    with tile.TileContext(nc) as tc:
        with (
            tc.tile_pool(name="xq", bufs=KT) as xq_p,
            tc.tile_pool(name="xk", bufs=KT) as xk_p,
            tc.tile_pool(name="xv", bufs=KT) as xv_p,
            tc.tile_pool(name="w", bufs=3) as w_p,
            tc.tile_pool(name="qk", bufs=2) as qk_p,
            tc.tile_pool(name="vext", bufs=NJT) as vext_p,
            tc.tile_pool(name="pt", bufs=pt_bufs) as pt_p,
            tc.tile_pool(name="small", bufs=10) as small_p,
            tc.tile_pool(name="ost", bufs=4) as ost_p,
            tc.tile_pool(name="sps", bufs=3, space="PSUM") as sps_p,
            tc.tile_pool(name="pps", bufs=2, space="PSUM") as pps_p,
        ):
            # ---- input DMAs (ordered: K-proj needs land first) ----
            def load_w(w_d):
                w_sb = w_p.tile([128, KT * GD], BF16, tag="w", name="w_sb")
                nc.sync.dma_start(
                    w_sb[:].rearrange("p (k o) -> p k o", k=KT),
                    w_d.ap().rearrange("(k p) o -> p k o", p=128))
                return w_sb

            wk_sb = load_w(wk_d)
            wq_sb = load_w(wq_d)
            mpad_sb = small_p.tile([128, NJT], BF16, tag="mpad")
            nc.sync.dma_start(mpad_sb[:], mpad_d.ap())
            # interleave K/Q k-tiles so both projections' k-loops advance
            # together behind the DMA stream; first column-halves land first
            # so the first score phase (i 0:1024, j 0:1024) starts early.
            xk_t = []
            xq_t = []
            jh = min(1024, J)
            for k in range(KT):
                t = xk_p.tile([128, J], BF16, tag="xk", name=f"xk{k}")
                nc.sync.dma_start(t[:, 0:jh],
                                  xk_d.ap()[k * 128:(k + 1) * 128, 0:jh])
                xk_t.append(t)
                t = xq_p.tile([128, S], BF16, tag="xq", name=f"xq{k}")
                nc.sync.dma_start(t[:, 0:1024],
                                  xq_d.ap()[k * 128:(k + 1) * 128, 0:1024])
                xq_t.append(t)
            for k in range(KT):
                if J > jh:
                    nc.sync.dma_start(xk_t[k][:, jh:J],
                                      xk_d.ap()[k * 128:(k + 1) * 128, jh:J])
                nc.sync.dma_start(xq_t[k][:, 1024:S],
                                  xq_d.ap()[k * 128:(k + 1) * 128, 1024:S])
            wv_sb = load_w(wv_d)
            xv_t = []
            for k in range(KT):
                t = xv_p.tile([128, J], BF16, tag="xv", name=f"xv{k}")
                nc.sync.dma_start(t[:], xv_d.ap()[k * 128:(k + 1) * 128, :])
                xv_t.append(t)
            if use_bias:
                bq_c = small_p.tile([128, 2], F32, tag="biasq")
                nc.sync.dma_start(
                    bq_c[:].rearrange("p (o x) -> p o x", o=2),
                    bq_d.ap().rearrange("(o p) x -> p o x", p=128))
                bk_c = small_p.tile([128, 2], F32, tag="biask")
                nc.sync.dma_start(
                    bk_c[:].rearrange("p (o x) -> p o x", o=2),
                    bk_d.ap().rearrange("(o p) x -> p o x", p=128))
                bv_row = small_p.tile([1, GD], BF16, tag="bvrow")
                nc.sync.dma_start(bv_row[:], bv_d.ap())
                ones_row = small_p.tile([1, 128], BF16, tag="ones")
                nc.vector.memset(ones_row[:], 1.0)

            # prime the ACT exp table during the initial DMA window
            warm = small_p.tile([1, 8], F32, tag="warm")
            nc.vector.memset(warm[:], 0.0)
            warm2 = small_p.tile([1, 8], F32, tag="warm2")
            nc.scalar.activation(warm2[:], warm[:], EXP, scale=1.0)

            # ---- projections ----
            qT = {}   # per otile: [128, S] bf16  (o on partitions)
            kTt = {}  # per otile: [128, J] bf16

            def proj_pass(x_tiles, w_sb, dst, bias_col, ot, chunk_group):
                """One k-outer accumulation pass over <=2 width-chunks.
                The 128 output dims are column-packed as two concurrent
                M=64 matmuls (tile_position (0,0)/(0,64)) sharing one rhs
                stream via separate XBUS groups."""
                ps = [pps_p.tile([128, 512], F32, tag="pps",
                                 name=f"pps{ot}{o0}") for (o0, _) in chunk_group]
                for k in range(KT):
                    lw = w_sb[:, k * GD + ot * 128:k * GD + (ot + 1) * 128]
                    for ci, (o0, n) in enumerate(chunk_group):
                        nc.tensor.matmul(ps[ci][:, 0:n], lhsT=lw,
                                         rhs=x_tiles[k][:, o0:o0 + n],
                                         start=(k == 0), stop=(k == KT - 1))
                for ci, (o0, n) in enumerate(chunk_group):
                    if use_bias:
                        nc.vector.tensor_scalar(
                            dst[:, o0:o0 + n], ps[ci][:, 0:n],
                            bias_col[:, ot:ot + 1], None,
                            mybir.AluOpType.add)
                    else:
                        nc.vector.tensor_copy(dst[:, o0:o0 + n], ps[ci][:, 0:n])

            def proj_passes(x_tiles, w_sb, dst_map, bias_col, width, ot):
                dst = qk_p.tile([128, width], BF16,
                                tag="qt" if width == S else "kt",
                                name=f"qk{ot}")
                dst_map[ot] = dst
                return [
            # returns one closure per single-chunk pass (filler-sized)
                    (lambda cg=cg: proj_pass(x_tiles, w_sb, dst, bias_col,
                                             ot, cg))
                    for cg in [[c] for c in _chunks(width, 512)]
                ]

            vext = [None] * NJT

            def v_wave(jts):
                """V projection (flipped orientation) for a couple of j-tiles
                + V_ext assembly."""
                ps = [pps_p.tile([128, GD], F32, tag="pps", name=f"ppsv{jt}")
                      for jt in jts]
                for k in range(KT):
                    for vi, jt in enumerate(jts):
                        nc.tensor.matmul(
                            ps[vi][:, :],
                            lhsT=xv_t[k][:, jt * 128:(jt + 1) * 128],
                            rhs=wv_sb[:, k * GD:(k + 1) * GD],
                            start=(k == 0),
                            stop=(k == KT - 1) and not use_bias)
                for vi, jt in enumerate(jts):
                    if use_bias:
                        nc.tensor.matmul(ps[vi][:, :], lhsT=ones_row[:, :],
                                         rhs=bv_row[:, :], start=False,
                                         stop=True)
                    ve = vext_p.tile([128, GH * VW1], BF16, tag="vext",
                                     name=f"vext{jt}")
                    for h in range(GH):
                        nc.vector.tensor_copy(
                            ve[:, h * VW1:h * VW1 + HD],
                            ps[vi][:, h * HD:(h + 1) * HD])
                        nc.vector.tensor_copy(
                            ve[:, h * VW1 + HD:h * VW1 + HD + 1],
                            mpad_sb[:, jt:jt + 1])
                    vext[jt] = ve

            def emit_qkt(ib, hp, fillers, start_jt=3):
                """Scores + exp for head pair hp of i-block ib. Emits one
                filler closure after each j-tile (from start_jt on) to keep
                PE fed while ACT chews on the exps; fillers too early would
                instead starve the score stream."""
                i0 = ib * IBLK
                pt = {}
                for jt in range(NJT):
                    psA = sps_p.tile([128, IBLK], F32, tag="sps",
                                     name=f"sA{ib}{hp}{jt}")
                    psB = sps_p.tile([128, IBLK], F32, tag="sps",
                                     name=f"sB{ib}{hp}{jt}")
                    for (o, n) in _chunks(IBLK, 512):
                        nc.tensor.matmul(
                            psA[:, o:o + n],
                            lhsT=kTt[hp][0:64, jt * 128:(jt + 1) * 128],
                            rhs=qT[hp][0:64, i0 + o:i0 + o + n],
                            start=True, stop=True)
                        nc.tensor.matmul(
                            psB[:, o:o + n],
                            lhsT=kTt[hp][64:128, jt * 128:(jt + 1) * 128],
                            rhs=qT[hp][64:128, i0 + o:i0 + o + n],
                            start=True, stop=True)
                    ptA = pt_p.tile([128, IBLK], BF16, tag="pt",
                                    name=f"ptA{ib}{hp}{jt}")
                    ptB = pt_p.tile([128, IBLK], BF16, tag="pt",
                                    name=f"ptB{ib}{hp}{jt}")
                    nc.scalar.activation(ptA[:], psA[:], EXP, scale=scale)
                    nc.scalar.activation(ptB[:], psB[:], EXP, scale=scale)
                    pt[(hp * 2, jt)] = ptA
                    pt[(hp * 2 + 1, jt)] = ptB
                    if jt >= start_jt and fillers:
                        fillers.pop(0)()
                while fillers:
                    fillers.pop(0)()
                return pt

            def pv_group(ib, hp, hl, icl, pt):
                """One PV accumulation group; numerators + denominator row
                DMA'd out unnormalized (host folds the division into
                unsharding)."""
                if True:
                    h = hp * 2 + hl
                    if True:
                        pv = pps_p.tile([VW1, 512], F32, tag="pps",
                                        name=f"pv{ib}{h}{icl}")
                        for jt in range(NJT):
                            nc.tensor.matmul(
                                pv[:, :],
                                lhsT=vext[jt][:, h * VW1:(h + 1) * VW1],
                                rhs=pt[(h, jt)][:, icl * 512:(icl + 1) * 512],
                                start=(jt == 0), stop=(jt == NJT - 1))
                        osb = ost_p.tile([VW1, 512], F32, tag="ost",
                                         name=f"o{ib}{h}{icl}")
                        nc.vector.tensor_copy(osb[:], pv[:, :])
                        nc.sync.dma_start(
                            out_d.ap()[h * VW1:(h + 1) * VW1,
                                       ib * IBLK + icl * 512:
                                       ib * IBLK + (icl + 1) * 512],
                            osb[:])

            # ---- emission schedule ----
            bqc = bq_c if use_bias else None
            bkc = bk_c if use_bias else None
            k0_passes = proj_passes(xk_t, wk_sb, kTt, bkc, J, 0)
            k0_passes[0]()   # j 0:512
            if len(k0_passes) > 1:
                k0_passes[1]()   # j 512:1024
            q0_passes = proj_passes(xq_t, wq_sb, qT, bqc, S, 0)
            q0_passes[0]()
            q0_passes[1]()   # i 0:1024 -> first score block can start

            v_fillers = [(lambda js=js: v_wave(js))
                         for js in _pairs(list(range(NJT)))]
            q1_fillers = proj_passes(xq_t, wq_sb, qT, bqc, S, 1)
            k1_fillers = proj_passes(xk_t, wk_sb, kTt, bkc, J, 1)

            def pv_fillers(ib, hp, pt):
                return [(lambda hl=hl, icl=icl: pv_group(ib, hp, hl, icl, pt))
                        for hl in range(2) for icl in range(IBLK // 512)]

            def pv_tail(ib, hp, pt):
                """Final-phase PV: interleave two groups' j-loops so their
                matmuls ride under the last exp window."""
                for hl in range(2):
                    h = hp * 2 + hl
                    pvs = []
                    for icl in range(IBLK // 512):
                        pv = pps_p.tile([VW1, 512], F32, tag="pps",
                                        name=f"pvt{ib}{h}{icl}")
                        pvs.append(pv)
                    for jt in range(NJT):
                        for icl in range(IBLK // 512):
                            nc.tensor.matmul(
                                pvs[icl][:, :],
                                lhsT=vext[jt][:, h * VW1:(h + 1) * VW1],
                                rhs=pt[(h, jt)][:, icl * 512:(icl + 1) * 512],
                                start=(jt == 0), stop=(jt == NJT - 1))
                    for icl in range(IBLK // 512):
                        osb = ost_p.tile([VW1, 512], F32, tag="ost",
                                         name=f"ot{ib}{h}{icl}")
                        nc.vector.tensor_copy(osb[:], pvs[icl][:, :])
                        nc.sync.dma_start(
                            out_d.ap()[h * VW1:(h + 1) * VW1,
                                       ib * IBLK + icl * 512:
                                       ib * IBLK + (icl + 1) * 512],
                            osb[:])

            if pipelined:
                # Front = K0/Q0 first halves only; every other matmul phase
                # runs as filler inside a later score phase's exp window.
                # PV groups of phase X release P^T buffers just in time for
                # phase X+2's allocations.
                pt00 = emit_qkt(0, 0,
                                k0_passes[2:] + q0_passes[2:]
                                + q1_fillers[:3])
                g00 = pv_fillers(0, 0, pt00)
                pt10 = emit_qkt(1, 0, q1_fillers[3:] + k1_fillers + v_fillers,
                                start_jt=2)
                g10 = pv_fillers(1, 0, pt10)
                pt01 = emit_qkt(0, 1, g00 + g10[:2], start_jt=1)
                g01 = pv_fillers(0, 1, pt01)
                pt11 = emit_qkt(1, 1, g10[2:] + g01, start_jt=1)
                for f in pv_fillers(1, 1, pt11):
                    f()
            else:
                for p in k0_passes[2:] + q0_passes[2:]:
                    p()
                pt00 = emit_qkt(0, 0, [])
                for f in v_fillers + q1_fillers + k1_fillers:
                    f()
                for f in pv_fillers(0, 0, pt00):
                    f()
                pt10 = emit_qkt(1, 0, [])
                for f in pv_fillers(1, 0, pt10):
                    f()
                pt01 = emit_qkt(0, 1, [])
                for f in pv_fillers(0, 1, pt01):
                    f()
                pt11 = emit_qkt(1, 1, [])
                for f in pv_fillers(1, 1, pt11):
                    f()

    nc.compile()
    return nc


def _prep_and_run(inputs, trace=False):
    query = np.asarray(inputs["query"], dtype=np.float32)
    key = np.asarray(inputs["key"], dtype=np.float32)
    value = np.asarray(inputs["value"], dtype=np.float32)
    mask = np.asarray(inputs["mask"]).reshape(S)
    w_q = np.asarray(inputs["w_q"], dtype=np.float32)
    b_q = np.asarray(inputs["b_q"], dtype=np.float32)
    w_k = np.asarray(inputs["w_k"], dtype=np.float32)
    b_k = np.asarray(inputs["b_k"], dtype=np.float32)
    w_v = np.asarray(inputs["w_v"], dtype=np.float32)
    b_v = np.asarray(inputs["b_v"], dtype=np.float32)

    use_bias = bool(np.any(b_q) or np.any(b_k) or np.any(b_v))

    # compact key/value over masked-out positions
    idx = np.nonzero(mask != 0)[0]
    J_real = int(len(idx))
    assert J_real > 0, "all positions masked: softmax undefined"
    J = max(512, ((J_real + 127) // 128) * 128)
    key_c = np.zeros((J, B, D), np.float32)
    key_c[:J_real] = key[idx]
    value_c = np.zeros((J, B, D), np.float32)
    value_c[:J_real] = value[idx]

    bf = ml_dtypes.bfloat16
    NJT = J // 128
    mflat = np.zeros(J, np.float32)
    mflat[:J_real] = 1  # mpad[p, t] = 1 iff t*128+p < J_real
    mpad = np.ascontiguousarray(mflat.reshape(NJT, 128).T).astype(bf)
    in_maps = []
    for core in range(N_CORES):
        b = core // GROUPS
        g = core % GROUPS
        hs = slice(g * GD, (g + 1) * GD)
        m = {
            "xq": np.ascontiguousarray(query[:, b, :].T).astype(bf),
            "xk": np.ascontiguousarray(key_c[:, b, :].T).astype(bf),
            "xv": np.ascontiguousarray(value_c[:, b, :].T).astype(bf),
            "wq": np.ascontiguousarray(w_q[hs, :].T).astype(bf),
            "wk": np.ascontiguousarray(w_k[hs, :].T).astype(bf),
            "wv": np.ascontiguousarray(w_v[hs, :].T).astype(bf),
            "mpad": mpad,
        }
        if use_bias:
            m["bq"] = np.ascontiguousarray(b_q[hs]).reshape(GD, 1)
            m["bk"] = np.ascontiguousarray(b_k[hs]).reshape(GD, 1)
            m["bv"] = np.ascontiguousarray(b_v[hs]).reshape(1, GD).astype(bf)
        in_maps.append(m)

    ck = (J, J_real, use_bias)
    if ck not in _CACHE:
        _CACHE[ck] = _build(J, J_real, use_bias)
    nc = _CACHE[ck]

    kwargs = {}
    if trace:
        kwargs = dict(trace=True, trace_cores=list(range(N_CORES)))
    res = run_bass_kernel_spmd(nc, in_maps, core_ids=list(range(N_CORES)),
                               **kwargs)

    out = np.empty((S, B, D), np.float32)
    for core in range(N_CORES):
        b = core // GROUPS
        g = core % GROUPS
        r = res.results[core]["out"].reshape(GH, VW1, S)
        out[:, b, g * GD:(g + 1) * GD] = (
            (r[:, :HD, :] / r[:, HD:HD + 1, :])     # softmax denominator
            .reshape(GD, S).T)
    return out, res


def kernel(**inputs):
    out, _ = _prep_and_run(inputs, trace=False)
    return out


def run_traced(**inputs):
    _, res = _prep_and_run(inputs, trace=True)
    return res


# revision 41
# speedup vs baseline: 1.1775x; 1.0011x over previous
"""MultiHeadAttention Trainium2 kernel.

Problem (hardcoded): S=2048, B=2, D=1024, H=16, HD=64, fp32 I/O.
  q = query @ w_q.T + b_q   (same for k, v), heads split from D
  scores[i,j,b,h] = (q_i . k_j)/8, masked where mask[j]==0, softmax over j
  out[i,b,:] = concat_h( sum_j p_ij v_j )

Sharding: 8 cores = 2 batches x 4 head-groups (4 heads / 256 dims each).
Host-side prep: cast to bf16, transpose to [D, seq] layout, and compact the
key/value sequence to the unmasked positions only (masked j contribute
exactly 0 after softmax), padded to a multiple of 128.

Per-core program (Tile framework), engineered so the ACT engine (exp) is
saturated from ~t=20us on and PE work hides beneath it:
  - Q,K projections k-outer (start as soon as the first k-tile lands),
    output qT/kT[o, i] with o (head dims) on partitions, bf16.
  - V projection flipped (x^T tiles stationary) giving V[j, o] with j on
    partitions; V_ext adds a per-head mask column (1 real / 0 padding) so
    the softmax denominator falls out of the PV matmul.
  - Scores transposed: S^T[j, i] = kT.T @ qT, two heads packed in the
    128-row PE array via base-partition row tiling (d=64 each).
  - P^T = exp(S^T / 8) on ACT in [128, 1024] chunks (bf16 out).
  - PV: out^T[vd, i] += V_ext[j,:].T @ P^T[j, i-chunk]  (stationary V_ext
    is tiny -> no LDWEIGHTS bottleneck), fp32 PSUM accumulation over j.
  - Output leaves unnormalized in [vd, i] orientation with the denominator
    rows; the host folds the softmax division + transpose into unsharding
    (4M flops of a 60-GFLOP problem).
  - Emission is software-pipelined: the only work ahead of the first score
    block is K/Q projection first-halves; V projection, remaining
    projections, and each phase's PV matmuls are "fillers" placed inside
    later phases' exp windows so the PE never starves the ACT engine.
"""

import math
import sys

sys.path.insert(0, "/opt/trn_rl_repo")

import numpy as np
import ml_dtypes

import concourse.tile as tile
from concourse import bacc, mybir
from concourse.bass_utils import run_bass_kernel_spmd

S, B, D, H, HD = 2048, 2, 1024, 16, 64
N_CORES = 8
GROUPS = 4          # head groups (cores per batch)
GH = H // GROUPS    # heads per core = 4
GD = GH * HD        # dims per core = 256
KT = D // 128       # contraction k-tiles = 8
IBLK = 1024         # i block (exp granularity / P^T tile width)
NIB = S // IBLK     # i blocks = 2
VW1 = HD + 1        # per-head vext width (64 v cols + denominator col)

BF16 = mybir.dt.bfloat16
F32 = mybir.dt.float32
F32R = mybir.dt.float32r
EXP = mybir.ActivationFunctionType.Exp

_CACHE = {}


def _chunks(total, step):
    out = []
    o = 0
    while o < total:
        n = min(step, total - o)
        out.append((o, n))
        o += n
    return out


def _pairs(seq):
    return [tuple(seq[i:i + 2]) for i in range(0, len(seq), 2)]


def _build(J, J_real, use_bias):
    """Build + compile the per-core Bass program (identical on all cores)."""
    NJT = J // 128
    nc = bacc.Bacc("TRN2", target_bir_lowering=False, debug=False,
                   enable_asserts=False)

    xq_d = nc.dram_tensor("xq", (D, S), BF16, kind="ExternalInput")
    xk_d = nc.dram_tensor("xk", (D, J), BF16, kind="ExternalInput")
    xv_d = nc.dram_tensor("xv", (D, J), BF16, kind="ExternalInput")
    wq_d = nc.dram_tensor("wq", (D, GD), BF16, kind="ExternalInput")
    wk_d = nc.dram_tensor("wk", (D, GD), BF16, kind="ExternalInput")
    wv_d = nc.dram_tensor("wv", (D, GD), BF16, kind="ExternalInput")
    mpad_d = nc.dram_tensor("mpad", (128, NJT), BF16, kind="ExternalInput")
    if use_bias:
        bq_d = nc.dram_tensor("bq", (GD, 1), F32, kind="ExternalInput")
        bk_d = nc.dram_tensor("bk", (GD, 1), F32, kind="ExternalInput")
        bv_d = nc.dram_tensor("bv", (1, GD), BF16, kind="ExternalInput")
    out_d = nc.dram_tensor("out", (GH * VW1, S), F32, kind="ExternalOutput")

    # SBUF budget for the P^T pool, in per-partition bytes (SBUF tiles
    # reserve their free-dim byte span on every partition).
    fixed_pp = (KT * S * 2                 # xq tiles
                + 2 * KT * J * 2           # xk, xv tiles
                + 3 * KT * GD * 2          # weights
                + 2 * S * 2 + 2 * J * 2    # qT/kT pool
                + NJT * (GH * VW1 + 8) * 2   # vext
                + 4 * 512 * 4              # out staging
                + 6 * 1024)                # consts, mpad, small, slack
    budget_pp = 188 * 1024 - fixed_pp
    pt_bufs = max(NJT + 2, min(4 * NJT + 4, budget_pp // (IBLK * 2)))
    # pipelined schedule holds two full score phases of P^T tiles
    pipelined = pt_bufs >= 4 * NJT

    scale = 1.0 / math.sqrt(HD)  # 0.125, folded into the exp

    with tile.TileContext(nc) as tc:
        with (
            tc.tile_pool(name="xq", bufs=KT) as xq_p,
            tc.tile_pool(name="xk", bufs=KT) as xk_p,
            tc.tile_pool(name="xv", bufs=KT) as xv_p,
            tc.tile_pool(name="w", bufs=3) as w_p,
            tc.tile_pool(name="qk", bufs=2) as qk_p,
            tc.tile_pool(name="vext", bufs=NJT) as vext_p,
            tc.tile_pool(name="pt", bufs=pt_bufs) as pt_p,
            tc.tile_pool(name="small", bufs=10) as small_p,
            tc.tile_pool(name="ost", bufs=4) as ost_p,
            tc.tile_pool(name="sps", bufs=3, space="PSUM") as sps_p,
            tc.tile_pool(name="pps", bufs=2, space="PSUM") as pps_p,
        ):
            # ---- input DMAs (ordered: K-proj needs land first) ----
            def load_w(w_d):
                w_sb = w_p.tile([128, KT * GD], BF16, tag="w", name="w_sb")
                nc.sync.dma_start(
                    w_sb[:].rearrange("p (k o) -> p k o", k=KT),
                    w_d.ap().rearrange("(k p) o -> p k o", p=128))
                return w_sb

            wk_sb = load_w(wk_d)
            wq_sb = load_w(wq_d)
            mpad_sb = small_p.tile([128, NJT], BF16, tag="mpad")
            nc.sync.dma_start(mpad_sb[:], mpad_d.ap())
            # interleave K/Q k-tiles so both projections' k-loops advance
            # together behind the DMA stream; first column-halves land first
            # so the first score phase (i 0:1024, j 0:1024) starts early.
            xk_t = []
            xq_t = []
            jh = min(1024, J)
            for k in range(KT):
                t = xk_p.tile([128, J], BF16, tag="xk", name=f"xk{k}")
                nc.sync.dma_start(t[:, 0:jh],
                                  xk_d.ap()[k * 128:(k + 1) * 128, 0:jh])
                xk_t.append(t)
                t = xq_p.tile([128, S], BF16, tag="xq", name=f"xq{k}")
                nc.sync.dma_start(t[:, 0:1024],
                                  xq_d.ap()[k * 128:(k + 1) * 128, 0:1024])
                xq_t.append(t)
            for k in range(KT):
                if J > jh:
                    nc.sync.dma_start(xk_t[k][:, jh:J],
                                      xk_d.ap()[k * 128:(k + 1) * 128, jh:J])
                nc.sync.dma_start(xq_t[k][:, 1024:S],
                                  xq_d.ap()[k * 128:(k + 1) * 128, 1024:S])
            wv_sb = load_w(wv_d)
            xv_t = []
            for k in range(KT):
                t = xv_p.tile([128, J], BF16, tag="xv", name=f"xv{k}")
                nc.sync.dma_start(t[:], xv_d.ap()[k * 128:(k + 1) * 128, :])
                xv_t.append(t)
            if use_bias:
                bq_c = small_p.tile([128, 2], F32, tag="biasq")
                nc.sync.dma_start(
                    bq_c[:].rearrange("p (o x) -> p o x", o=2),
                    bq_d.ap().rearrange("(o p) x -> p o x", p=128))
                bk_c = small_p.tile([128, 2], F32, tag="biask")
                nc.sync.dma_start(
                    bk_c[:].rearrange("p (o x) -> p o x", o=2),
                    bk_d.ap().rearrange("(o p) x -> p o x", p=128))
                bv_row = small_p.tile([1, GD], BF16, tag="bvrow")
                nc.sync.dma_start(bv_row[:], bv_d.ap())
                ones_row = small_p.tile([1, 128], BF16, tag="ones")
                nc.vector.memset(ones_row[:], 1.0)

            # prime the ACT exp table during the initial DMA window
            warm = small_p.tile([1, 8], F32, tag="warm")
            nc.vector.memset(warm[:], 0.0)
            warm2 = small_p.tile([1, 8], F32, tag="warm2")
            nc.scalar.activation(warm2[:], warm[:], EXP, scale=1.0)

            # ---- projections ----
            qT = {}   # per otile: [128, S] bf16  (o on partitions)
            kTt = {}  # per otile: [128, J] bf16

            def proj_pass(x_tiles, w_sb, dst, bias_col, ot, chunk_group):
                """One k-outer accumulation pass over <=2 width-chunks.
                The 128 output dims are column-packed as two concurrent
                M=64 matmuls (tile_position (0,0)/(0,64)) sharing one rhs
                stream via separate XBUS groups."""
                ps = [pps_p.tile([128, 512], F32, tag="pps",
                                 name=f"pps{ot}{o0}") for (o0, _) in chunk_group]
                for k in range(KT):
                    lw = w_sb[:, k * GD + ot * 128:k * GD + (ot + 1) * 128]
                    for ci, (o0, n) in enumerate(chunk_group):
                        nc.tensor.matmul(ps[ci][:, 0:n], lhsT=lw,
                                         rhs=x_tiles[k][:, o0:o0 + n],
                                         start=(k == 0), stop=(k == KT - 1))
                for ci, (o0, n) in enumerate(chunk_group):
                    if use_bias:
                        nc.vector.tensor_scalar(
                            dst[:, o0:o0 + n], ps[ci][:, 0:n],
                            bias_col[:, ot:ot + 1], None,
                            mybir.AluOpType.add)
                    else:
                        nc.vector.tensor_copy(dst[:, o0:o0 + n], ps[ci][:, 0:n])

            def proj_passes(x_tiles, w_sb, dst_map, bias_col, width, ot):
                dst = qk_p.tile([128, width], BF16,
                                tag="qt" if width == S else "kt",
                                name=f"qk{ot}")
                dst_map[ot] = dst
                return [
            # returns one closure per single-chunk pass (filler-sized)
                    (lambda cg=cg: proj_pass(x_tiles, w_sb, dst, bias_col,
                                             ot, cg))
                    for cg in [[c] for c in _chunks(width, 512)]
                ]

            vext = [None] * NJT

            def v_wave(jts):
                """V projection (flipped orientation) for a couple of j-tiles
                + V_ext assembly."""
                ps = [pps_p.tile([128, GD], F32, tag="pps", name=f"ppsv{jt}")
                      for jt in jts]
                for k in range(KT):
                    for vi, jt in enumerate(jts):
                        nc.tensor.matmul(
                            ps[vi][:, :],
                            lhsT=xv_t[k][:, jt * 128:(jt + 1) * 128],
                            rhs=wv_sb[:, k * GD:(k + 1) * GD],
                            start=(k == 0),
                            stop=(k == KT - 1) and not use_bias)
                for vi, jt in enumerate(jts):
                    if use_bias:
                        nc.tensor.matmul(ps[vi][:, :], lhsT=ones_row[:, :],
                                         rhs=bv_row[:, :], start=False,
                                         stop=True)
                    ve = vext_p.tile([128, GH * VW1], BF16, tag="vext",
                                     name=f"vext{jt}")
                    for h in range(GH):
                        nc.vector.tensor_copy(
                            ve[:, h * VW1:h * VW1 + HD],
                            ps[vi][:, h * HD:(h + 1) * HD])
                        nc.vector.tensor_copy(
                            ve[:, h * VW1 + HD:h * VW1 + HD + 1],
                            mpad_sb[:, jt:jt + 1])
                    vext[jt] = ve

            def emit_qkt(ib, hp, fillers, start_jt=3):
                """Scores + exp for head pair hp of i-block ib. Emits one
                filler closure after each j-tile (from start_jt on) to keep
                PE fed while ACT chews on the exps; fillers too early would
                instead starve the score stream."""
                i0 = ib * IBLK
                pt = {}
                for jt in range(NJT):
                    psA = sps_p.tile([128, IBLK], F32, tag="sps",
                                     name=f"sA{ib}{hp}{jt}")
                    psB = sps_p.tile([128, IBLK], F32, tag="sps",
                                     name=f"sB{ib}{hp}{jt}")
                    for (o, n) in _chunks(IBLK, 512):
                        nc.tensor.matmul(
                            psA[:, o:o + n],
                            lhsT=kTt[hp][0:64, jt * 128:(jt + 1) * 128],
                            rhs=qT[hp][0:64, i0 + o:i0 + o + n],
                            start=True, stop=True)
                        nc.tensor.matmul(
                            psB[:, o:o + n],
                            lhsT=kTt[hp][64:128, jt * 128:(jt + 1) * 128],
                            rhs=qT[hp][64:128, i0 + o:i0 + o + n],
                            start=True, stop=True)
                    ptA = pt_p.tile([128, IBLK], BF16, tag="pt",
                                    name=f"ptA{ib}{hp}{jt}")
                    ptB = pt_p.tile([128, IBLK], BF16, tag="pt",
                                    name=f"ptB{ib}{hp}{jt}")
                    nc.scalar.activation(ptA[:], psA[:], EXP, scale=scale)
                    nc.scalar.activation(ptB[:], psB[:], EXP, scale=scale)
                    pt[(hp * 2, jt)] = ptA
                    pt[(hp * 2 + 1, jt)] = ptB
                    if jt >= start_jt and fillers:
                        fillers.pop(0)()
                while fillers:
                    fillers.pop(0)()
                return pt

            def pv_group(ib, hp, hl, icl, pt):
                """One PV accumulation group; numerators + denominator row
                DMA'd out unnormalized (host folds the division into
                unsharding)."""
                if True:
                    h = hp * 2 + hl
                    if True:
                        pv = pps_p.tile([VW1, 512], F32, tag="pps",
                                        name=f"pv{ib}{h}{icl}")
                        for jt in range(NJT):
                            nc.tensor.matmul(
                                pv[:, :],
                                lhsT=vext[jt][:, h * VW1:(h + 1) * VW1],
                                rhs=pt[(h, jt)][:, icl * 512:(icl + 1) * 512],
                                start=(jt == 0), stop=(jt == NJT - 1))
                        osb = ost_p.tile([VW1, 512], F32, tag="ost",
                                         name=f"o{ib}{h}{icl}")
                        nc.vector.tensor_copy(osb[:], pv[:, :])
                        nc.sync.dma_start(
                            out_d.ap()[h * VW1:(h + 1) * VW1,
                                       ib * IBLK + icl * 512:
                                       ib * IBLK + (icl + 1) * 512],
                            osb[:])

            # ---- emission schedule ----
            bqc = bq_c if use_bias else None
            bkc = bk_c if use_bias else None
            k0_passes = proj_passes(xk_t, wk_sb, kTt, bkc, J, 0)
            k0_passes[0]()   # j 0:512
            if len(k0_passes) > 1:
                k0_passes[1]()   # j 512:1024
            q0_passes = proj_passes(xq_t, wq_sb, qT, bqc, S, 0)
            q0_passes[0]()
            q0_passes[1]()   # i 0:1024 -> first score block can start

            v_fillers = [(lambda js=js: v_wave(js))
                         for js in _pairs(list(range(NJT)))]
            q1_fillers = proj_passes(xq_t, wq_sb, qT, bqc, S, 1)
            k1_fillers = proj_passes(xk_t, wk_sb, kTt, bkc, J, 1)

            def pv_fillers(ib, hp, pt):
                return [(lambda hl=hl, icl=icl: pv_group(ib, hp, hl, icl, pt))
                        for hl in range(2) for icl in range(IBLK // 512)]

            if pipelined:
                # Front = K0/Q0 first halves only; every other matmul phase
                # runs as filler inside a later score phase's exp window.
                # PV groups of phase X release P^T buffers just in time for
                # phase X+2's allocations.
                pt00 = emit_qkt(0, 0,
                                k0_passes[2:] + q0_passes[2:]
                                + q1_fillers[:3])
                g00 = pv_fillers(0, 0, pt00)
                pt10 = emit_qkt(1, 0, q1_fillers[3:] + k1_fillers + v_fillers,
                                start_jt=2)
                g10 = pv_fillers(1, 0, pt10)
                pt01 = emit_qkt(0, 1, g00 + g10[:2], start_jt=1)
                g01 = pv_fillers(0, 1, pt01)
                pt11 = emit_qkt(1, 1, g10[2:] + g01, start_jt=1)
                for f in pv_fillers(1, 1, pt11):
                    f()
            else:
                for p in k0_passes[2:] + q0_passes[2:]:
                    p()
                pt00 = emit_qkt(0, 0, [])
                for f in v_fillers + q1_fillers + k1_fillers:
                    f()
                for f in pv_fillers(0, 0, pt00):
                    f()
                pt10 = emit_qkt(1, 0, [])
                for f in pv_fillers(1, 0, pt10):
                    f()
                pt01 = emit_qkt(0, 1, [])
                for f in pv_fillers(0, 1, pt01):
                    f()
                pt11 = emit_qkt(1, 1, [])
                for f in pv_fillers(1, 1, pt11):
                    f()

    nc.compile()
    return nc


def _prep_and_run(inputs, trace=False):
    query = np.asarray(inputs["query"], dtype=np.float32)
    key = np.asarray(inputs["key"], dtype=np.float32)
    value = np.asarray(inputs["value"], dtype=np.float32)
    mask = np.asarray(inputs["mask"]).reshape(S)
    w_q = np.asarray(inputs["w_q"], dtype=np.float32)
    b_q = np.asarray(inputs["b_q"], dtype=np.float32)
    w_k = np.asarray(inputs["w_k"], dtype=np.float32)
    b_k = np.asarray(inputs["b_k"], dtype=np.float32)
    w_v = np.asarray(inputs["w_v"], dtype=np.float32)
    b_v = np.asarray(inputs["b_v"], dtype=np.float32)

    use_bias = bool(np.any(b_q) or np.any(b_k) or np.any(b_v))

    # compact key/value over masked-out positions
    idx = np.nonzero(mask != 0)[0]
    J_real = int(len(idx))
    assert J_real > 0, "all positions masked: softmax undefined"
    J = max(512, ((J_real + 127) // 128) * 128)
    key_c = np.zeros((J, B, D), np.float32)
    key_c[:J_real] = key[idx]
    value_c = np.zeros((J, B, D), np.float32)
    value_c[:J_real] = value[idx]

    bf = ml_dtypes.bfloat16
    NJT = J // 128
    mflat = np.zeros(J, np.float32)
    mflat[:J_real] = 1  # mpad[p, t] = 1 iff t*128+p < J_real
    mpad = np.ascontiguousarray(mflat.reshape(NJT, 128).T).astype(bf)
    in_maps = []
    for core in range(N_CORES):
        b = core // GROUPS
        g = core % GROUPS
        hs = slice(g * GD, (g + 1) * GD)
        m = {
            "xq": np.ascontiguousarray(query[:, b, :].T).astype(bf),
            "xk": np.ascontiguousarray(key_c[:, b, :].T).astype(bf),
            "xv": np.ascontiguousarray(value_c[:, b, :].T).astype(bf),
            "wq": np.ascontiguousarray(w_q[hs, :].T).astype(bf),
            "wk": np.ascontiguousarray(w_k[hs, :].T).astype(bf),
            "wv": np.ascontiguousarray(w_v[hs, :].T).astype(bf),
            "mpad": mpad,
        }
        if use_bias:
            m["bq"] = np.ascontiguousarray(b_q[hs]).reshape(GD, 1)
            m["bk"] = np.ascontiguousarray(b_k[hs]).reshape(GD, 1)
            m["bv"] = np.ascontiguousarray(b_v[hs]).reshape(1, GD).astype(bf)
        in_maps.append(m)

    ck = (J, J_real, use_bias)
    if ck not in _CACHE:
        _CACHE[ck] = _build(J, J_real, use_bias)
    nc = _CACHE[ck]

    kwargs = {}
    if trace:
        kwargs = dict(trace=True, trace_cores=list(range(N_CORES)))
    res = run_bass_kernel_spmd(nc, in_maps, core_ids=list(range(N_CORES)),
                               **kwargs)

    out = np.empty((S, B, D), np.float32)
    for core in range(N_CORES):
        b = core // GROUPS
        g = core % GROUPS
        r = res.results[core]["out"].reshape(GH, VW1, S)
        out[:, b, g * GD:(g + 1) * GD] = (
            (r[:, :HD, :] / r[:, HD:HD + 1, :])     # softmax denominator
            .reshape(GD, S).T)
    return out, res


def kernel(**inputs):
    out, _ = _prep_and_run(inputs, trace=False)
    return out


def run_traced(**inputs):
    _, res = _prep_and_run(inputs, trace=True)
    return res
